# revision 1
# baseline (speedup 1.0000x reference)
"""Chamfer distance kernel for 8 Trainium2 NeuronCores (Bass/Tile).

Problem: pred/target (4, 8192, 3) fp32 -> scalar mean chamfer distance
(bidirectional nearest-neighbor squared distances, mean over batch).

Sharding (data parallel on batch x pred-half): core c handles batch
b = c // 2 and pred-half h = c % 2 (4096 of the 8192 pred points) against
ALL 8192 targets of that batch. Forward mins (over targets) complete per
core; backward row-mins (over preds) are per-half partials that the host
min-combines across the core pair.

Device math per core:
  d2[m, n] = ||q_m||^2 + ||p_n||^2 - 2 q_m . p_n   (m target, n pred)
as ONE K=13 matmul per tile using bf16 hi/lo splitting (~2^-18 relative
error; bf16 streams 1 PE cycle/row where fp32 needs 4):
    Q_aug rows: [qh0..2, qh0..2, ql0..2, q2h, q2l, 1, 1]  (q* = split(-2q))
    P_aug rows: [ph0..2, pl0..2, ph0..2, 1,  1,  p2h, p2l]
  dot = qh.ph + qh.pl + ql.ph + q2 + p2 ~= -2 q.p + ||q||^2 + ||p||^2.

Pipeline per target chunk mi (128 targets on PSUM partitions, all 4096
preds on the free axis, 8 matmuls of [13,128]x[13,512] into two 4-bank
PSUM groups, double buffered). All reductions run in NEGATED space
(values are -d2) so that every reduction is a max and GPSIMD's
partition_all_reduce(max) — its only ordering op — applies directly:
  - ScalarE (sole PSUM reader -> no cross-engine PSUM bank serialization)
    copies each group, scaled by -1 (free in the ACT copy), into one
    contiguous [128, 4096] bf16 staging tile.
  - VectorE runs the big ops at the bf16 2x tensor_tensor rate:
      * forward: A = max(A, staged) in one [128, 4096] op (A bf16),
      * backward: fold the two n-groups (valid under the row-reduce),
        max-halve 2048 -> 256 at 2x, then the 1x-rate tensor_reduce
        produces this mi's row-reduce column r[:, mi] (fp32 = -rowmin).
    The DVE is the bottleneck engine at ~300us/core (TimelineSim ~321us
    end to end; the naive fp32 all-DVE variant models at 628us).
  - Tail: GPSIMD partition_all_reduce(max) collapses A across partitions
    so only a [1, 4096] fp32 row ships to the host.

The reference's maximum(d2, 0) clamp commutes with min, so the host
applies it after all mins. The host negates, does the cross-core
combines and means in float64, and returns the fp32 scalar.

bf16 rounding of d2 before the min reductions adds ~1e-4 relative error
to the final scalar (validated ~1e-5 on both PRNG variants of the
reference inputs); the matmul's hi/lo split error is ~2^-18 per term.
Hardware notes for this environment: tensor_tensor_reduce crashes at
execution and tensor_tensor/min-max DMA-CCE/most Q7 ops fail walrus'
opcode-on-engine check, which is why the kernel restricts itself to
matmul / ACT copy / DVE TT+reduce / partition_all_reduce / memset.
"""

import functools

import numpy as np
import ml_dtypes

import concourse.bacc as bacc
import concourse.bass_isa as bass_isa
import concourse.mybir as mybir
import concourse.tile as tile

BF16 = ml_dtypes.bfloat16

B = 4            # batches
N = 8192         # points per cloud
NCORES = 8
NH = N // 2      # preds per core (4096)
K = 13           # augmented contraction dim
MI = N // 128    # 64 target chunks of 128
GF = 2048        # free elements per psum group (4 banks)
NG = NH // GF    # 2 groups per mi
BIG = 3.0e38


def _split_bf16(x):
    """fp32 -> (hi, lo) bf16 pair with x ~= hi + lo (error ~2^-18 |x|)."""
    xh = x.astype(BF16)
    xl = (x - xh.astype(np.float32)).astype(BF16)
    return xh, xl


def _aug_inputs(pred, target):
    """Per-core augmented bf16 matrices: {"q_aug": [13, 8192], "p_aug": [13, 4096]}."""
    in_maps = []
    for c in range(NCORES):
        b, h = divmod(c, 2)
        q = np.asarray(target[b], dtype=np.float32)              # (8192, 3)
        p = np.asarray(pred[b][h * NH:(h + 1) * NH], dtype=np.float32)

        qh, ql = _split_bf16(-2.0 * q)
        q2h, q2l = _split_bf16(np.sum(q * q, axis=-1, dtype=np.float32))
        onesq = np.ones(N, dtype=BF16)
        q_aug = np.stack([
            qh[:, 0], qh[:, 1], qh[:, 2],
            qh[:, 0], qh[:, 1], qh[:, 2],
            ql[:, 0], ql[:, 1], ql[:, 2],
            q2h, q2l, onesq, onesq,
        ])                                                       # (13, 8192)

        ph, pl = _split_bf16(p)
        p2h, p2l = _split_bf16(np.sum(p * p, axis=-1, dtype=np.float32))
        onesp = np.ones(NH, dtype=BF16)
        p_aug = np.stack([
            ph[:, 0], ph[:, 1], ph[:, 2],
            pl[:, 0], pl[:, 1], pl[:, 2],
            ph[:, 0], ph[:, 1], ph[:, 2],
            onesp, onesp, p2h, p2l,
        ])                                                       # (13, 4096)
        in_maps.append({"q_aug": np.ascontiguousarray(q_aug),
                        "p_aug": np.ascontiguousarray(p_aug)})
    return in_maps


@functools.lru_cache(maxsize=4)
def _build_program(mi_count=MI, mode="bf16fold"):
    """mode "alldve": fp32 reduce + min-accumulate straight from PSUM (no
    bf16 rounding, ~2x slower). mode "bf16fold": the pipeline described in
    the module docstring."""
    nc = bacc.Bacc("TRN2", target_bir_lowering=False, debug=False,
                   num_devices=NCORES)
    f32 = mybir.dt.float32
    bf16 = mybir.dt.bfloat16
    mn = mybir.AluOpType.min
    mx = mybir.AluOpType.max

    a_dt = f32 if mode == "alldve" else bf16
    r_cols = NG * MI if mode == "alldve" else MI
    a_rows = 128 if mode == "alldve" else 1

    q_dram = nc.dram_tensor("q_aug", [K, N], bf16, kind="ExternalInput")
    p_dram = nc.dram_tensor("p_aug", [K, NH], bf16, kind="ExternalInput")
    a_dram = nc.dram_tensor("a_out", [a_rows, NH],
                            f32 if mode == "bf16fold" else a_dt,
                            kind="ExternalOutput")
    r_dram = nc.dram_tensor("r_out", [128, r_cols], f32, kind="ExternalOutput")

    with tile.TileContext(nc) as tc:
        with tc.tile_pool(name="const", bufs=1) as cpool, \
             tc.tile_pool(name="stage", bufs=3) as spool, \
             tc.tile_pool(name="fold", bufs=2) as fpool, \
             tc.tile_pool(name="psum", bufs=2, space="PSUM") as ppool:
            q_sb = cpool.tile([K, N], bf16)
            p_sb = cpool.tile([K, NH], bf16)
            a_sb = cpool.tile([128, NH], a_dt)
            r_sb = cpool.tile([128, r_cols], f32)

            nc.sync.dma_start(out=q_sb[:], in_=q_dram.ap())
            nc.sync.dma_start(out=p_sb[:], in_=p_dram.ap())
            nc.gpsimd.memset(a_sb[:], BIG if mode == "alldve" else -BIG)

            for mi in range(mi_count):
                s_sb = None
                for g in range(NG):
                    ps = ppool.tile([128, GF], f32, tag="ps")
                    for j in range(GF // 512):
                        nj = (GF // 512) * g + j
                        nc.tensor.matmul(
                            ps[:, j * 512:(j + 1) * 512],
                            q_sb[:, mi * 128:(mi + 1) * 128],
                            p_sb[:, nj * 512:(nj + 1) * 512],
                            start=True, stop=True,
                        )
                    if mode == "alldve":
                        asl = a_sb[:, g * GF:(g + 1) * GF]
                        nc.vector.tensor_reduce(
                            out=r_sb[:, g * MI + mi: g * MI + mi + 1],
                            in_=ps[:], axis=mybir.AxisListType.X, op=mn)
                        nc.vector.tensor_tensor(
                            out=asl, in0=ps[:], in1=asl, op=mn)
                    elif mode == "bf16fold":
                        # stage as NEGATED bf16 (scale=-1 is free in the
                        # ACT copy); all reductions below become max, so
                        # partition_all_reduce(max) applies directly.
                        if s_sb is None:
                            s_sb = spool.tile([128, NG * GF], bf16,
                                              tag="stage")
                        nc.scalar.mul(s_sb[:, g * GF:(g + 1) * GF], ps[:],
                                      -1.0)
                    else:
                        raise ValueError(mode)
                if mode == "bf16fold":
                    # forward max-accumulate, both groups in one 2x bf16 op
                    nc.vector.tensor_tensor(
                        out=a_sb[:], in0=s_sb[:], in1=a_sb[:], op=mx)
                    # backward: fold the two n-groups (valid under the
                    # row-reduce), max-halve at 2x, then the 1x-rate reduce
                    fold = fpool.tile([128, GF], bf16, tag="fold")
                    nc.vector.tensor_tensor(
                        out=fold[:], in0=s_sb[:, :GF], in1=s_sb[:, GF:],
                        op=mx)
                    w = GF // 2
                    while w >= 256:
                        nc.vector.tensor_tensor(
                            out=fold[:, :w], in0=fold[:, :w],
                            in1=fold[:, w:2 * w], op=mx)
                        w //= 2
                    nc.vector.tensor_reduce(
                        out=r_sb[:, mi: mi + 1], in_=fold[:, :2 * w],
                        axis=mybir.AxisListType.X, op=mx)

            if mode == "bf16fold":
                # forward finalize on device: all-reduce max across
                # partitions (GPSIMD), so only a [1, 4096] row ships to
                # the host (fmin = -row, values are negated d2).
                pm_sb = cpool.tile([128, NH], f32)
                for hh in range(2):
                    sl = slice(hh * NH // 2, (hh + 1) * NH // 2)
                    nc.gpsimd.partition_all_reduce(
                        pm_sb[:, sl], a_sb[:, sl], 128,
                        bass_isa.ReduceOp.max)
                nc.sync.dma_start(out=a_dram.ap(), in_=pm_sb[0:1, :])
            else:
                nc.sync.dma_start(out=a_dram.ap(), in_=a_sb[:])
            nc.sync.dma_start(out=r_dram.ap(), in_=r_sb[:])

    nc.compile()
    return nc


# ---------------------------------------------------------------------------
# Execution: a cached jitted shard_map runner over the 8 axon devices
# (rebuilding it per call would re-trace and cost ~0.5s/call), with a
# fallback to the stock run_bass_kernel_spmd path.
# ---------------------------------------------------------------------------

_RUNNER_CACHE = {}


def _make_runner(nc):
    import jax
    from jax.sharding import Mesh, PartitionSpec
    from jax.experimental.shard_map import shard_map
    from concourse import bass2jax
    from concourse.bass2jax import _bass_exec_p, install_neuronx_cc_hook

    install_neuronx_cc_hook()
    partition_name = nc.partition_id_tensor.name if nc.partition_id_tensor else None
    in_names, out_names, out_avals, zero_shapes = [], [], [], []
    for alloc in nc.m.functions[0].allocations:
        if not isinstance(alloc, mybir.MemoryLocationSet):
            continue
        name = alloc.memorylocations[0].name
        if alloc.kind == "ExternalInput":
            if name != partition_name:
                in_names.append(name)
        elif alloc.kind == "ExternalOutput":
            np_dtype = mybir.dt.np(alloc.dtype)
            shape = tuple(alloc.tensor_shape)
            out_names.append(name)
            out_avals.append(jax.core.ShapedArray(shape, np_dtype))
            zero_shapes.append((shape, np_dtype))

    n_params, n_outs = len(in_names), len(out_avals)
    all_in_names = list(in_names) + list(out_names)
    if partition_name is not None:
        all_in_names.append(partition_name)
    donate = tuple(range(n_params, n_params + n_outs))

    def _body(*args):
        operands = list(args)
        if partition_name is not None:
            operands.append(bass2jax.partition_id_tensor())
        outs = _bass_exec_p.bind(
            *operands, out_avals=tuple(out_avals),
            in_names=tuple(all_in_names), out_names=tuple(out_names),
            lowering_input_output_aliases=(),
            sim_require_finite=True, sim_require_nnan=True, nc=nc)
        return tuple(outs)

    devices = jax.devices()[:NCORES]
    mesh = Mesh(np.asarray(devices), ("core",))
    del donate  # outputs are fully written by the kernel; skip donation so
    # the zero "output seed" buffers can stay resident on device across calls
    sharded = jax.jit(
        shard_map(_body, mesh=mesh,
                  in_specs=(PartitionSpec("core"),) * (n_params + n_outs),
                  out_specs=(PartitionSpec("core"),) * n_outs,
                  check_rep=False),
        keep_unused=True)
    from jax.sharding import NamedSharding
    sh = NamedSharding(mesh, PartitionSpec("core"))
    zeros_dev = [
        jax.device_put(np.zeros((NCORES * s[0], *s[1:]), d), sh)
        for s, d in zero_shapes]

    def run(in_maps):
        concat_in = [
            np.concatenate([np.asarray(in_maps[c][name])
                            for c in range(NCORES)], axis=0)
            for name in in_names]
        outs = sharded(*concat_in, *zeros_dev)
        return [
            {name: np.asarray(outs[i]).reshape(NCORES, *out_avals[i].shape)[c]
             for i, name in enumerate(out_names)}
            for c in range(NCORES)]

    return run


def _run_spmd(nc, in_maps):
    key = id(nc)
    try:
        if key not in _RUNNER_CACHE:
            _RUNNER_CACHE[key] = _make_runner(nc)
        return _RUNNER_CACHE[key](in_maps)
    except Exception:
        from concourse.bass_utils import run_bass_kernel_spmd
        return run_bass_kernel_spmd(
            nc, in_maps, core_ids=list(range(NCORES))).results


def _host_reduce(results):
    """Combine per-core outputs into the final scalar (float64 internally)."""
    chamfers = []
    for b in range(B):
        fs = []
        bvecs = []
        for h in range(2):
            res = results[2 * b + h]
            A = np.asarray(res["a_out"]).astype(np.float64)   # [1|128, 4096]
            R = np.asarray(res["r_out"]).astype(np.float64)   # [128, 64|128]
            if A.shape[0] == 1:                               # device-reduced
                fs.append(-A[0])                              # fmin = -max(-d2)
            else:
                fs.append(A.min(axis=0))                      # [4096]
            if R.shape[1] == 2 * MI:                          # alldve layout
                R = np.minimum(R[:, :MI], R[:, MI:])          # [128, 64]
            else:
                R = -R                                        # negated space
            bvecs.append(R.T.reshape(N))                      # m = 128*mi + p
        f = np.maximum(np.concatenate(fs), 0.0)               # [8192] fwd mins
        bv = np.maximum(np.minimum(bvecs[0], bvecs[1]), 0.0)  # [8192] bwd mins
        chamfers.append(f.mean() + bv.mean())
    return np.float32(np.mean(chamfers))


def kernel(pred, target):
    pred = np.asarray(pred, dtype=np.float32)
    target = np.asarray(target, dtype=np.float32)
    in_maps = _aug_inputs(pred, target)
    nc = _build_program()
    results = _run_spmd(nc, in_maps)
    return np.array(_host_reduce(results), dtype=np.float32)



# revision 13
# speedup vs baseline: 1.2340x; 1.2340x over previous
"""Chamfer distance kernel for 8 Trainium2 NeuronCores (Bass/Tile).

Problem: pred/target (4, 8192, 3) fp32 -> scalar mean chamfer distance
(bidirectional nearest-neighbor squared distances, mean over batch).

Sharding (data parallel on batch x pred-half): core c handles batch
b = c // 2 and pred-half h = c % 2 (4096 of the 8192 pred points) against
ALL 8192 targets of that batch. Forward mins (over targets) complete per
core; backward row-mins (over preds) are per-half partials that the host
min-combines across the core pair.

Device math per core (one K=13 matmul per tile, bf16 hi/lo split,
~2^-18 relative error): psum[m, n] = -d2[m, n] -- the NEGATION is folded
into the matmul by negating the whole augmented q matrix on the host, so
every reduction everywhere is a MAX (gpsimd partition_all_reduce's only
ordering op) with no staging-time scale needed.

Per target-chunk mi (128 targets on PSUM partitions, 4096 preds on the
free axis, 8 matmuls into two 4-bank PSUM groups, double buffered), one
of three pipeline classes chosen to balance ACT / DVE / GPSIMD:

  class a (ACT+DVE, the old pipeline):
    ACT copies both PSUM groups into a [128, 4096] bf16 staging tile;
    DVE max-accumulates A[128, 4096] (forward) and runs the fold tree +
    reduce for the backward row-mins r[:, mi].
  class b (ACT+GPSIMD fwd, DVE bwd):
    ACT stages as in (a); GPSIMD partition_all_reduce(max) collapses the
    staged tile's 128 target-partitions -> this chunk's forward partial
    row, parked in its own partition of a row-stack tile; DVE only runs
    the backward fold tree.
  class c (no ACT):
    the otherwise-idle DMA engines drain both PSUM groups into an fp32
    SBUF scratch (PSUM turnaround ~5us independent of any engine
    backlog -- reading PSUM directly from GPSIMD parks the banks behind
    Pool's in-order queue and stalls the PE); GPSIMD
    partition_all_reduce(max) then takes the forward row from the
    scratch and DVE's backward fold level-1 is a 1x TT max over the two
    fp32 scratch halves with the tree continuing in bf16. ACT is not
    involved at all -- this class is what pulls ACT below its 242us
    all-staging floor.

The forward finalization (min over A's 128 partitions and over the
stacked b/c rows) and all cross-core combines happen on the host in
float64; the device ships A [128, 4096] bf16, the row stack
[n_b+n_c, 4096] bf16, and r [128, 64] f32 raw. The reference's
maximum(d2, 0) clamp commutes with min and is applied on the host.

Engine budget per core (TimelineSim cost model, 64 chunks):
  ACT  = 3784 * (a+b)            (1892ns per [128,2048] PSUM->SBUF copy)
  DVE  = 4763a + 2570b + 3633c   (TT bf16 2x, fp32 1x, reduce 1x)
  GPS  = 5784b + 5879c           (partition_all_reduce ~1.39ns/elem)
with a+b+c=64; (a,b,c)=(25,32,7) balances all three at ~216-227us vs
the all-DVE baseline's 305us DVE wall.

Hardware notes for this environment: tensor_tensor_reduce crashes at
execution and tensor_tensor/min-max DMA-CCE/most Q7 ops fail walrus'
opcode-on-engine check, which is why the kernel restricts itself to
matmul / ACT copy / DVE TT+reduce / partition_all_reduce / memset / DMA.
"""

import functools

import numpy as np
import ml_dtypes

import concourse.bacc as bacc
import concourse.bass_isa as bass_isa
import concourse.mybir as mybir
import concourse.tile as tile

BF16 = ml_dtypes.bfloat16

B = 4            # batches
N = 8192         # points per cloud
NCORES = 8
NH = N // 2      # preds per core (4096)
K = 13           # augmented contraction dim
MI = N // 128    # 64 target chunks of 128
GF = 2048        # free elements per psum group (4 banks)
NG = NH // GF    # 2 groups per mi
BIG = 3.0e38

# chunk-class mix: (n_a, n_b, n_c) summing to MI; see module docstring.
# c-chunks measure net-negative in TimelineSim (each one's PSUM-bank
# residency behind the Pool/DVE in-order queues stalls the PE ~12us,
# which also drops the PE out of its p-state ramp), so the shipped mix
# uses none; classes a/b balance ACT~242 / DVE~240 / GPS~226us.
CLASS_MIX = (25, 39, 0)


def _split_bf16(x):
    """fp32 -> (hi, lo) bf16 pair with x ~= hi + lo (error ~2^-18 |x|)."""
    xh = x.astype(BF16)
    xl = (x - xh.astype(np.float32)).astype(BF16)
    return xh, xl


def _aug_inputs(pred, target):
    """Per-core augmented bf16 matrices: {"q_aug": [13, 8192], "p_aug": [13, 4096]}.

    q_aug is NEGATED so that psum = -d2 directly (see module docstring).
    """
    in_maps = []
    for c in range(NCORES):
        b, h = divmod(c, 2)
        q = np.asarray(target[b], dtype=np.float32)              # (8192, 3)
        p = np.asarray(pred[b][h * NH:(h + 1) * NH], dtype=np.float32)

        qh, ql = _split_bf16(-2.0 * q)
        q2h, q2l = _split_bf16(np.sum(q * q, axis=-1, dtype=np.float32))
        onesq = np.ones(N, dtype=BF16)
        q_aug = -np.stack([
            qh[:, 0], qh[:, 1], qh[:, 2],
            qh[:, 0], qh[:, 1], qh[:, 2],
            ql[:, 0], ql[:, 1], ql[:, 2],
            q2h, q2l, onesq, onesq,
        ])                                                       # (13, 8192)

        ph, pl = _split_bf16(p)
        p2h, p2l = _split_bf16(np.sum(p * p, axis=-1, dtype=np.float32))
        onesp = np.ones(NH, dtype=BF16)
        p_aug = np.stack([
            ph[:, 0], ph[:, 1], ph[:, 2],
            pl[:, 0], pl[:, 1], pl[:, 2],
            ph[:, 0], ph[:, 1], ph[:, 2],
            onesp, onesp, p2h, p2l,
        ])                                                       # (13, 4096)
        in_maps.append({"q_aug": np.ascontiguousarray(q_aug),
                        "p_aug": np.ascontiguousarray(p_aug)})
    return in_maps


def _class_pattern(mix):
    """Class order across the MI chunks. Each c-chunk is placed directly
    after a run of a-chunks: a-chunks enqueue no Pool work, so the Pool
    engine's strictly in-order queue is drained when the c-chunk's
    PSUM-reading partition_all_reduce arrives -- otherwise the c-chunk's
    PSUM banks park behind queued b-reduces and stall the PE."""
    na, nb, nc_ = mix
    assert na + nb + nc_ == MI
    if nc_ == 0:
        # interleave a/b evenly
        pat = []
        acca = accb = 0.0
        for _ in range(MI):
            acca += na / MI
            accb += nb / MI
            if acca >= accb:
                pat.append("a"); acca -= 1.0
            else:
                pat.append("b"); accb -= 1.0
        return pat
    pat = []
    for u in range(nc_):
        ka = na * (u + 1) // nc_ - na * u // nc_
        kb = nb * (u + 1) // nc_ - nb * u // nc_
        pat.extend(["b"] * kb + ["a"] * ka + ["c"])
    return pat


def _gps_ar(nc, out_ap, in_ap, channels=128):
    """partition_all_reduce(max); out_ap may have fewer partitions than
    channels (direct single-row output), which the python wrapper's assert
    forbids but the ISA lowering accepts."""
    gp = nc.gpsimd
    _in = gp.lower_ap(in_ap, for_isa=True)
    _out = gp.lower_ap(out_ap, for_isa=True)
    return gp.add_instruction(
        bass_isa.InstPartitionAllReduce(
            name=f"I-{gp.bass.next_id()}",
            ins=[_in],
            outs=[_out],
            _channels=channels,
            _reduce_op=bass_isa.ReduceOp.max,
        ))


@functools.lru_cache(maxsize=8)
def _build_program(mix=CLASS_MIX, direct_out=False):
    """direct_out=True: gpsimd writes its forward row straight into the
    stack tile's row (1-partition out AP); False: full-width scratch +
    a row DMA (safe fallback)."""
    nc = bacc.Bacc("TRN2", target_bir_lowering=False, debug=False,
                   num_devices=NCORES)
    f32 = mybir.dt.float32
    bf16 = mybir.dt.bfloat16
    mx = mybir.AluOpType.max

    pattern = _class_pattern(mix)
    n_bc = sum(1 for p in pattern if p in ("b", "c"))

    q_dram = nc.dram_tensor("q_aug", [K, N], bf16, kind="ExternalInput")
    p_dram = nc.dram_tensor("p_aug", [K, NH], bf16, kind="ExternalInput")
    a_dram = nc.dram_tensor("a_out", [128, NH], bf16, kind="ExternalOutput")
    r_dram = nc.dram_tensor("r_out", [128, MI], f32, kind="ExternalOutput")
    s_dram = (nc.dram_tensor("s_out", [n_bc, NH], bf16, kind="ExternalOutput")
              if n_bc else None)

    with tile.TileContext(nc) as tc:
        with tc.tile_pool(name="const", bufs=1) as cpool, \
             tc.tile_pool(name="stage", bufs=4) as spool, \
             tc.tile_pool(name="fold", bufs=2) as fpool, \
             tc.tile_pool(name="scr", bufs=2) as gpool, \
             tc.tile_pool(name="scrf", bufs=2) as gfpool, \
             tc.tile_pool(name="psum", bufs=2, space="PSUM") as ppool:
            q_sb = cpool.tile([K, N], bf16)
            p_sb = cpool.tile([K, NH], bf16)
            a_sb = cpool.tile([128, NH], bf16)
            r_sb = cpool.tile([128, MI], f32)
            stack = None
            if n_bc:
                stack = cpool.tile([n_bc, NH], bf16, name="stack")

            nc.sync.dma_start(out=q_sb[:], in_=q_dram.ap())
            nc.sync.dma_start(out=p_sb[:], in_=p_dram.ap())
            nc.gpsimd.memset(a_sb[:], -BIG)

            def bwd_tail(fold, mi):
                """fold [128, GF] bf16 -> r_sb[:, mi] (max = -rowmin)."""
                w = GF // 2
                while w >= 256:
                    nc.vector.tensor_tensor(
                        out=fold[:, :w], in0=fold[:, :w],
                        in1=fold[:, w:2 * w], op=mx)
                    w //= 2
                nc.vector.tensor_reduce(
                    out=r_sb[:, mi: mi + 1], in_=fold[:, :2 * w],
                    axis=mybir.AxisListType.X, op=mx)

            row = 0
            for mi, cls in enumerate(pattern):
                ps_tiles = []
                s_sb = scr = None
                if cls in ("a", "b"):
                    s_sb = spool.tile([128, NH], bf16, tag="stage", name="s_sb")
                if cls in ("b", "c") and not direct_out:
                    scr = gpool.tile([128, NH], bf16, tag="scr", name="scr")
                for g in range(NG):
                    ps = ppool.tile([128, GF], f32, tag="ps")
                    for j in range(GF // 512):
                        nj = (GF // 512) * g + j
                        nc.tensor.matmul(
                            ps[:, j * 512:(j + 1) * 512],
                            q_sb[:, mi * 128:(mi + 1) * 128],
                            p_sb[:, nj * 512:(nj + 1) * 512],
                            start=True, stop=True,
                        )
                    ps_tiles.append(ps)
                    gsl = slice(g * GF, (g + 1) * GF)
                    if cls in ("a", "b"):
                        nc.scalar.mul(s_sb[:, gsl], ps[:], 1.0)
                    else:  # c: forward straight from PSUM on gpsimd.
                        # high_priority: the PSUM banks are held until this
                        # runs, so it must jump the Pool queue's backlog or
                        # the PE stalls on the bank rotation.
                        with tc.high_priority():
                            if direct_out:
                                _gps_ar(nc, stack[row:row + 1, gsl], ps[:])
                            else:
                                _gps_ar(nc, scr[:, gsl], ps[:])

                if cls == "a":
                    # forward max-accumulate, both groups in one 2x bf16 op
                    nc.vector.tensor_tensor(
                        out=a_sb[:], in0=s_sb[:], in1=a_sb[:], op=mx)
                elif cls == "b":
                    # forward on gpsimd from the staged tile
                    if direct_out:
                        _gps_ar(nc, stack[row:row + 1, :], s_sb[:])
                    else:
                        _gps_ar(nc, scr[:], s_sb[:])

                # backward fold level 1 (valid under the row-reduce)
                fold = fpool.tile([128, GF], bf16, tag="fold")
                if cls == "c":
                    # high_priority: last PSUM consumer of the c-chunk (see
                    # the partition_all_reduce comment above).
                    with tc.high_priority():
                        nc.vector.tensor_tensor(
                            out=fold[:], in0=ps_tiles[0][:],
                            in1=ps_tiles[1][:], op=mx)
                else:
                    nc.vector.tensor_tensor(
                        out=fold[:], in0=s_sb[:, :GF], in1=s_sb[:, GF:],
                        op=mx)
                bwd_tail(fold, mi)

                if cls in ("b", "c"):
                    if not direct_out:
                        nc.sync.dma_start(out=stack[row:row + 1, :],
                                          in_=scr[0:1, :])
                    row += 1

            nc.sync.dma_start(out=a_dram.ap(), in_=a_sb[:])
            nc.sync.dma_start(out=r_dram.ap(), in_=r_sb[:])
            if n_bc:
                nc.sync.dma_start(out=s_dram.ap(), in_=stack[:])

    nc.compile()
    return nc


# ---------------------------------------------------------------------------
# Execution: a cached jitted shard_map runner over the 8 axon devices
# (rebuilding it per call would re-trace and cost ~0.5s/call), with a
# fallback to the stock run_bass_kernel_spmd path.
# ---------------------------------------------------------------------------

_RUNNER_CACHE = {}


def _make_runner(nc):
    import jax
    from jax.sharding import Mesh, PartitionSpec
    from jax.experimental.shard_map import shard_map
    from concourse import bass2jax
    from concourse.bass2jax import _bass_exec_p, install_neuronx_cc_hook

    install_neuronx_cc_hook()
    partition_name = nc.partition_id_tensor.name if nc.partition_id_tensor else None
    in_names, out_names, out_avals, zero_shapes = [], [], [], []
    for alloc in nc.m.functions[0].allocations:
        if not isinstance(alloc, mybir.MemoryLocationSet):
            continue
        name = alloc.memorylocations[0].name
        if alloc.kind == "ExternalInput":
            if name != partition_name:
                in_names.append(name)
        elif alloc.kind == "ExternalOutput":
            np_dtype = mybir.dt.np(alloc.dtype)
            shape = tuple(alloc.tensor_shape)
            out_names.append(name)
            out_avals.append(jax.core.ShapedArray(shape, np_dtype))
            zero_shapes.append((shape, np_dtype))

    n_params, n_outs = len(in_names), len(out_avals)
    all_in_names = list(in_names) + list(out_names)
    if partition_name is not None:
        all_in_names.append(partition_name)

    def _body(*args):
        operands = list(args)
        if partition_name is not None:
            operands.append(bass2jax.partition_id_tensor())
        outs = _bass_exec_p.bind(
            *operands, out_avals=tuple(out_avals),
            in_names=tuple(all_in_names), out_names=tuple(out_names),
            lowering_input_output_aliases=(),
            sim_require_finite=True, sim_require_nnan=True, nc=nc)
        return tuple(outs)

    devices = jax.devices()[:NCORES]
    mesh = Mesh(np.asarray(devices), ("core",))
    # outputs are fully written by the kernel; skip donation so the zero
    # "output seed" buffers can stay resident on device across calls
    sharded = jax.jit(
        shard_map(_body, mesh=mesh,
                  in_specs=(PartitionSpec("core"),) * (n_params + n_outs),
                  out_specs=(PartitionSpec("core"),) * n_outs,
                  check_rep=False),
        keep_unused=True)
    from jax.sharding import NamedSharding
    sh = NamedSharding(mesh, PartitionSpec("core"))
    zeros_dev = [
        jax.device_put(np.zeros((NCORES * s[0], *s[1:]), d), sh)
        for s, d in zero_shapes]

    def run(in_maps):
        concat_in = [
            np.concatenate([np.asarray(in_maps[c][name])
                            for c in range(NCORES)], axis=0)
            for name in in_names]
        outs = sharded(*concat_in, *zeros_dev)
        return [
            {name: np.asarray(outs[i]).reshape(NCORES, *out_avals[i].shape)[c]
             for i, name in enumerate(out_names)}
            for c in range(NCORES)]

    return run


def _run_spmd(nc, in_maps):
    key = id(nc)
    try:
        if key not in _RUNNER_CACHE:
            _RUNNER_CACHE[key] = _make_runner(nc)
        return _RUNNER_CACHE[key](in_maps)
    except Exception:
        from concourse.bass_utils import run_bass_kernel_spmd
        return run_bass_kernel_spmd(
            nc, in_maps, core_ids=list(range(NCORES))).results


def _host_reduce(results):
    """Combine per-core outputs into the final scalar (float64 internally).

    All device values are in NEGATED space (-d2): forward min per pred =
    -max over A partitions and stacked b/c rows; backward row-min = -r.
    """
    chamfers = []
    for b in range(B):
        fs = []
        bvecs = []
        for h in range(2):
            res = results[2 * b + h]
            A = np.asarray(res["a_out"]).astype(np.float64)   # [128, 4096]
            fmax = A.max(axis=0)
            if "s_out" in res:
                S = np.asarray(res["s_out"]).astype(np.float64)
                fmax = np.maximum(fmax, S.max(axis=0))
            fs.append(-fmax)                                  # [4096] fwd mins
            R = np.asarray(res["r_out"]).astype(np.float64)   # [128, 64]
            bvecs.append(-R.T.reshape(N))                     # m = 128*mi + p
        f = np.maximum(np.concatenate(fs), 0.0)               # [8192] fwd mins
        bv = np.maximum(np.minimum(bvecs[0], bvecs[1]), 0.0)  # [8192] bwd mins
        chamfers.append(f.mean() + bv.mean())
    return np.float32(np.mean(chamfers))


def kernel(pred, target):
    pred = np.asarray(pred, dtype=np.float32)
    target = np.asarray(target, dtype=np.float32)
    in_maps = _aug_inputs(pred, target)
    nc = _build_program()
    results = _run_spmd(nc, in_maps)
    return np.array(_host_reduce(results), dtype=np.float32)


# revision 32
# speedup vs baseline: 1.2630x; 1.0235x over previous
"""Chamfer distance kernel for 8 Trainium2 NeuronCores (Bass/Tile).

Problem: pred/target (4, 8192, 3) fp32 -> scalar mean chamfer distance
(bidirectional nearest-neighbor squared distances, mean over batch).

Sharding (data parallel on batch x pred-half): core c handles batch
b = c // 2 and pred-half h = c % 2 (4096 of the 8192 pred points) against
ALL 8192 targets of that batch. Forward mins (over targets) complete per
core; backward row-mins (over preds) are per-half partials that the host
min-combines across the core pair.

Device math per core (one K=13 matmul per tile, bf16 hi/lo split,
~2^-18 relative error): psum[m, n] = -d2[m, n] -- the NEGATION is folded
into the matmul by negating the whole augmented q matrix on the host, so
every reduction everywhere is a MAX (gpsimd partition_all_reduce's only
ordering op) with no staging-time scale needed.

Per target-chunk mi (128 targets on PSUM partitions, 4096 preds on the
free axis, 8 matmuls into two 4-bank PSUM groups, double buffered), one
of three pipeline classes chosen to balance ACT / DVE / GPSIMD:

  class a (ACT+DVE, the old pipeline):
    ACT copies both PSUM groups into a [128, 4096] bf16 staging tile;
    DVE max-accumulates A[128, 4096] (forward) and runs the fold tree +
    reduce for the backward row-mins r[:, mi].
  class b (ACT+GPSIMD fwd, DVE bwd):
    ACT stages as in (a); GPSIMD partition_all_reduce(max) collapses the
    staged tile's 128 target-partitions -> this chunk's forward partial
    row, parked in its own partition of a row-stack tile; DVE only runs
    the backward fold tree.
  class c (no ACT):
    the otherwise-idle DMA engines drain both PSUM groups into an fp32
    SBUF scratch (PSUM turnaround ~5us independent of any engine
    backlog -- reading PSUM directly from GPSIMD parks the banks behind
    Pool's in-order queue and stalls the PE); GPSIMD
    partition_all_reduce(max) then takes the forward row from the
    scratch and DVE's backward fold level-1 is a 1x TT max over the two
    fp32 scratch halves with the tree continuing in bf16. ACT is not
    involved at all -- this class is what pulls ACT below its 242us
    all-staging floor.

The forward finalization (min over A's 128 partitions and over the
stacked b/c rows) and all cross-core combines happen on the host in
float64; the device ships A [128, 4096] bf16, the row stack
[n_b+n_c, 4096] bf16, and r [128, 64] f32 raw. The reference's
maximum(d2, 0) clamp commutes with min and is applied on the host.

Engine budget per core (TimelineSim cost model, 64 chunks):
  ACT  = 3784 * (a+b)            (1892ns per [128,2048] PSUM->SBUF copy)
  DVE  = 4763a + 2570b + 3633c   (TT bf16 2x, fp32 1x, reduce 1x)
  GPS  = 5784b + 5879c           (partition_all_reduce ~1.39ns/elem)
with a+b+c=64; (a,b,c)=(25,32,7) balances all three at ~216-227us vs
the all-DVE baseline's 305us DVE wall.

Hardware notes for this environment: tensor_tensor_reduce crashes at
execution and tensor_tensor/min-max DMA-CCE/most Q7 ops fail walrus'
opcode-on-engine check, which is why the kernel restricts itself to
matmul / ACT copy / DVE TT+reduce / partition_all_reduce / memset / DMA.
"""

import functools

import numpy as np
import ml_dtypes

import concourse.bacc as bacc
import concourse.bass_isa as bass_isa
import concourse.mybir as mybir
import concourse.tile as tile

BF16 = ml_dtypes.bfloat16

B = 4            # batches
N = 8192         # points per cloud
NCORES = 8
NH = N // 2      # preds per core (4096)
K = 13           # augmented contraction dim
MI = N // 128    # 64 target chunks of 128
GF = 2048        # free elements per psum group (4 banks)
NG = NH // GF    # 2 groups per mi
BIG = 3.0e38

# chunk-class mix: (n_a, n_b, n_d) summing to MI; see docstring.
# d-chunks are a-chunks whose second PSUM group is staged by DVE
# instead of ACT (tensor_scalar copy), trading 1892ns ACT for 2258ns
# DVE to shave the ACT wall. (A "c" class -- gpsimd reading PSUM
# directly, no staging -- measured net-negative in TimelineSim: the
# PSUM-bank residency behind the Pool/DVE in-order queues stalls the
# PE ~12us per chunk and drops it out of its p-state ramp.)
CLASS_MIX = (18, 38, 8)


def _split_bf16(x):
    """fp32 -> (hi, lo) bf16 pair with x ~= hi + lo (error ~2^-18 |x|)."""
    xh = x.astype(BF16)
    xl = (x - xh.astype(np.float32)).astype(BF16)
    return xh, xl


def _aug_inputs(pred, target):
    """Per-core augmented bf16 matrices: {"q_aug": [13, 8192], "p_aug": [13, 4096]}.

    q_aug is NEGATED so that psum = -d2 directly (see module docstring).
    """
    in_maps = []
    for c in range(NCORES):
        b, h = divmod(c, 2)
        q = np.asarray(target[b], dtype=np.float32)              # (8192, 3)
        p = np.asarray(pred[b][h * NH:(h + 1) * NH], dtype=np.float32)

        qh, ql = _split_bf16(-2.0 * q)
        q2h, q2l = _split_bf16(np.sum(q * q, axis=-1, dtype=np.float32))
        onesq = np.ones(N, dtype=BF16)
        q_aug = -np.stack([
            qh[:, 0], qh[:, 1], qh[:, 2],
            qh[:, 0], qh[:, 1], qh[:, 2],
            ql[:, 0], ql[:, 1], ql[:, 2],
            q2h, q2l, onesq, onesq,
        ])                                                       # (13, 8192)

        ph, pl = _split_bf16(p)
        p2h, p2l = _split_bf16(np.sum(p * p, axis=-1, dtype=np.float32))
        onesp = np.ones(NH, dtype=BF16)
        p_aug = np.stack([
            ph[:, 0], ph[:, 1], ph[:, 2],
            pl[:, 0], pl[:, 1], pl[:, 2],
            ph[:, 0], ph[:, 1], ph[:, 2],
            onesp, onesp, p2h, p2l,
        ])                                                       # (13, 4096)
        in_maps.append({"q_aug": np.ascontiguousarray(q_aug),
                        "p_aug": np.ascontiguousarray(p_aug)})
    return in_maps


def _pair_pattern(mix):
    """Pair-granular class order over MI//2 chunk-pairs. Mostly "ab"
    pairs (classes alternate within the pair, so ACT/Pool/DVE load stays
    smooth chunk-to-chunk), "bb" pairs spread evenly to absorb the b
    surplus, and one quiet "aa" tail pair so the slow Pool/DMA tails of
    the last b-chunks overlap ACT's final copies instead of extending
    past them. d-chunks (second PSUM group staged by DVE instead of ACT)
    replace the a of evenly spaced ab-pairs. na+nb+nd=64 keeps parity
    automatic: A=na+nd and nb are both even or both odd is impossible
    (A + nb = 64), so nb - (A-2) is always even."""
    na, nb, nd = mix
    assert na + nb + nd == MI
    A = na + nd                       # a-family chunks
    n_ab = A - 2                      # alternating ab pairs
    assert n_ab >= 0 and (nb - n_ab) % 2 == 0
    n_bb = (nb - n_ab) // 2
    assert n_bb >= 0
    pairs = []
    acc = 0.0
    for _ in range(n_ab):
        pairs.append("ab")
        acc += n_bb / max(n_ab, 1)
        while acc >= 1.0:
            pairs.append("bb"); acc -= 1.0
    while len(pairs) < MI // 2 - 1:
        pairs.append("bb")
    pairs.append("aa")                # quiet tail
    assert len(pairs) == MI // 2
    if nd:
        ab_pos = [i for i, p in enumerate(pairs) if p == "ab"]
        assert len(ab_pos) >= nd
        step = max(1, len(ab_pos) // nd)
        for i in ab_pos[::step][:nd]:
            pairs[i] = "db"
    assert sum(p.count("b") for p in pairs) == nb
    assert sum(p.count("d") for p in pairs) == nd
    return pairs


def _gps_ar(nc, out_ap, in_ap, channels=128):
    """partition_all_reduce(max); out_ap may have fewer partitions than
    channels (direct single-row output), which the python wrapper's assert
    forbids but the ISA lowering accepts."""
    gp = nc.gpsimd
    _in = gp.lower_ap(in_ap, for_isa=True)
    _out = gp.lower_ap(out_ap, for_isa=True)
    return gp.add_instruction(
        bass_isa.InstPartitionAllReduce(
            name=f"I-{gp.bass.next_id()}",
            ins=[_in],
            outs=[_out],
            _channels=channels,
            _reduce_op=bass_isa.ReduceOp.max,
        ))


@functools.lru_cache(maxsize=8)
def _build_program(mix=CLASS_MIX):
    nc = bacc.Bacc("TRN2", target_bir_lowering=False, debug=False,
                   num_devices=NCORES)
    f32 = mybir.dt.float32
    bf16 = mybir.dt.bfloat16
    mx = mybir.AluOpType.max

    pairs = _pair_pattern(mix)
    n_bc = sum(p.count("b") for p in pairs)

    q_dram = nc.dram_tensor("q_aug", [K, N], bf16, kind="ExternalInput")
    p_dram = nc.dram_tensor("p_aug", [K, NH], bf16, kind="ExternalInput")
    a_dram = nc.dram_tensor("a_out", [128, NH], bf16, kind="ExternalOutput")
    r_dram = nc.dram_tensor("r_out", [128, MI], f32, kind="ExternalOutput")
    s_dram = (nc.dram_tensor("s_out", [n_bc, NH], bf16, kind="ExternalOutput")
              if n_bc else None)

    with tile.TileContext(nc) as tc:
        with tc.tile_pool(name="const", bufs=1) as cpool, \
             tc.tile_pool(name="stage", bufs=4) as spool, \
             tc.tile_pool(name="fold", bufs=2) as fpool, \
             tc.tile_pool(name="scr", bufs=2) as gpool, \
             tc.tile_pool(name="psum", bufs=2, space="PSUM") as ppool:
            q_sb = cpool.tile([K, N], bf16)
            p_sb = cpool.tile([K, NH], bf16)
            a_sb = cpool.tile([128, NH], bf16)
            r_sb = cpool.tile([128, MI], f32)

            # p first, then q in two pieces so chunk 0's matmuls (which
            # need only q[:, :128]) start ~1.5us sooner
            nc.sync.dma_start(out=p_sb[:], in_=p_dram.ap())
            nc.sync.dma_start(out=q_sb[:, :1024], in_=q_dram.ap()[:, :1024])
            nc.sync.dma_start(out=q_sb[:, 1024:], in_=q_dram.ap()[:, 1024:])
            nc.gpsimd.memset(a_sb[:], -BIG)

            # PE p-state warm-up: ~3us of throwaway matmuls on the first
            # loaded input so the ramp (LOW->MID->full clock after 3us of
            # continuous execution) is paid on junk work, letting chunk
            # 0's real matmuls run at the full 2.4GHz
            wm = ppool.tile([128, GF], f32, tag="ps", name="wm")
            for r in range(2):
                for j in range(GF // 512):
                    nc.tensor.matmul(
                        wm[:, j * 512:(j + 1) * 512],
                        p_sb[:, 0:128], p_sb[:, 0:512],
                        start=True, stop=True,
                    )

            row = 0
            for pi, pcls in enumerate(pairs):
                # --- per chunk: fill, stage, forward, backward head ---
                # (all per-chunk so work starts as soon as that chunk is
                # staged; only the short tree tail is fused pair-wise)
                fold = fpool.tile([128, 2 * GF], bf16, tag="fold")
                for k in range(2):
                    mi = 2 * pi + k
                    cls = pcls[k]
                    s_sb = spool.tile([128, NH], bf16, tag="stage",
                                      name="s_sb")
                    for g in range(NG):
                        ps = ppool.tile([128, GF], f32, tag="ps")
                        for j in range(GF // 512):
                            nj = (GF // 512) * g + j
                            nc.tensor.matmul(
                                ps[:, j * 512:(j + 1) * 512],
                                q_sb[:, mi * 128:(mi + 1) * 128],
                                p_sb[:, nj * 512:(nj + 1) * 512],
                                start=True, stop=True,
                            )
                        gsl = slice(g * GF, (g + 1) * GF)
                        if cls == "d" and g == 1:
                            # DVE stages this group: max(x, -BIG) is a
                            # dtype-converting copy on the vector engine
                            nc.vector.tensor_scalar_max(
                                out=s_sb[:, gsl], in0=ps[:], scalar1=-BIG)
                        else:
                            nc.scalar.mul(s_sb[:, gsl], ps[:], 1.0)

                    if cls == "b":
                        scr = gpool.tile([128, NH], bf16, tag="scr",
                                         name="scr")
                        _gps_ar(nc, scr[:], s_sb[:])
                        nc.sync.dma_start(
                            out=s_dram.ap()[row:row + 1, :],
                            in_=scr[0:1, :])
                        row += 1
                    else:
                        nc.vector.tensor_tensor(
                            out=a_sb[:], in0=s_sb[:], in1=a_sb[:], op=mx)
                    # backward fold level 1 + first halving
                    nc.vector.tensor_tensor(
                        out=fold[:, k * GF:(k + 1) * GF],
                        in0=s_sb[:, :GF], in1=s_sb[:, GF:], op=mx)
                    nc.vector.tensor_tensor(
                        out=fold[:, k * GF:k * GF + GF // 2],
                        in0=fold[:, k * GF:k * GF + GF // 2],
                        in1=fold[:, k * GF + GF // 2:(k + 1) * GF], op=mx)

                # --- fused pair tree tail -----------------------------
                fv = fold[:].rearrange("p (c h) -> p c h", c=2, h=GF)
                w = GF // 4
                while w >= 256:
                    nc.vector.tensor_tensor(
                        out=fv[:, :, :w], in0=fv[:, :, :w],
                        in1=fv[:, :, w:2 * w], op=mx)
                    w //= 2
                nc.vector.tensor_reduce(
                    out=r_sb[:, 2 * pi: 2 * pi + 2], in_=fv[:, :, :2 * w],
                    axis=mybir.AxisListType.X, op=mx)

            nc.sync.dma_start(out=a_dram.ap(), in_=a_sb[:])
            nc.sync.dma_start(out=r_dram.ap(), in_=r_sb[:])

    nc.compile()
    return nc


# ---------------------------------------------------------------------------
# Execution: a cached jitted shard_map runner over the 8 axon devices
# (rebuilding it per call would re-trace and cost ~0.5s/call), with a
# fallback to the stock run_bass_kernel_spmd path.
# ---------------------------------------------------------------------------

_RUNNER_CACHE = {}


def _make_runner(nc):
    import jax
    from jax.sharding import Mesh, PartitionSpec
    from jax.experimental.shard_map import shard_map
    from concourse import bass2jax
    from concourse.bass2jax import _bass_exec_p, install_neuronx_cc_hook

    install_neuronx_cc_hook()
    partition_name = nc.partition_id_tensor.name if nc.partition_id_tensor else None
    in_names, out_names, out_avals, zero_shapes = [], [], [], []
    for alloc in nc.m.functions[0].allocations:
        if not isinstance(alloc, mybir.MemoryLocationSet):
            continue
        name = alloc.memorylocations[0].name
        if alloc.kind == "ExternalInput":
            if name != partition_name:
                in_names.append(name)
        elif alloc.kind == "ExternalOutput":
            np_dtype = mybir.dt.np(alloc.dtype)
            shape = tuple(alloc.tensor_shape)
            out_names.append(name)
            out_avals.append(jax.core.ShapedArray(shape, np_dtype))
            zero_shapes.append((shape, np_dtype))

    n_params, n_outs = len(in_names), len(out_avals)
    all_in_names = list(in_names) + list(out_names)
    if partition_name is not None:
        all_in_names.append(partition_name)

    def _body(*args):
        operands = list(args)
        if partition_name is not None:
            operands.append(bass2jax.partition_id_tensor())
        outs = _bass_exec_p.bind(
            *operands, out_avals=tuple(out_avals),
            in_names=tuple(all_in_names), out_names=tuple(out_names),
            lowering_input_output_aliases=(),
            sim_require_finite=True, sim_require_nnan=True, nc=nc)
        return tuple(outs)

    devices = jax.devices()[:NCORES]
    mesh = Mesh(np.asarray(devices), ("core",))
    # outputs are fully written by the kernel; skip donation so the zero
    # "output seed" buffers can stay resident on device across calls
    sharded = jax.jit(
        shard_map(_body, mesh=mesh,
                  in_specs=(PartitionSpec("core"),) * (n_params + n_outs),
                  out_specs=(PartitionSpec("core"),) * n_outs,
                  check_rep=False),
        keep_unused=True)
    from jax.sharding import NamedSharding
    sh = NamedSharding(mesh, PartitionSpec("core"))
    zeros_dev = [
        jax.device_put(np.zeros((NCORES * s[0], *s[1:]), d), sh)
        for s, d in zero_shapes]

    def run(in_maps):
        concat_in = [
            np.concatenate([np.asarray(in_maps[c][name])
                            for c in range(NCORES)], axis=0)
            for name in in_names]
        outs = sharded(*concat_in, *zeros_dev)
        return [
            {name: np.asarray(outs[i]).reshape(NCORES, *out_avals[i].shape)[c]
             for i, name in enumerate(out_names)}
            for c in range(NCORES)]

    return run


def _run_spmd(nc, in_maps):
    key = id(nc)
    try:
        if key not in _RUNNER_CACHE:
            _RUNNER_CACHE[key] = _make_runner(nc)
        return _RUNNER_CACHE[key](in_maps)
    except Exception:
        from concourse.bass_utils import run_bass_kernel_spmd
        return run_bass_kernel_spmd(
            nc, in_maps, core_ids=list(range(NCORES))).results


def _host_reduce(results):
    """Combine per-core outputs into the final scalar (float64 internally).

    All device values are in NEGATED space (-d2): forward min per pred =
    -max over A partitions and stacked b/c rows; backward row-min = -r.
    """
    chamfers = []
    for b in range(B):
        fs = []
        bvecs = []
        for h in range(2):
            res = results[2 * b + h]
            A = np.asarray(res["a_out"]).astype(np.float64)   # [128, 4096]
            fmax = A.max(axis=0)
            if "s_out" in res:
                S = np.asarray(res["s_out"]).astype(np.float64)
                fmax = np.maximum(fmax, S.max(axis=0))
            fs.append(-fmax)                                  # [4096] fwd mins
            R = np.asarray(res["r_out"]).astype(np.float64)   # [128, 64]
            bvecs.append(-R.T.reshape(N))                     # m = 128*mi + p
        f = np.maximum(np.concatenate(fs), 0.0)               # [8192] fwd mins
        bv = np.maximum(np.minimum(bvecs[0], bvecs[1]), 0.0)  # [8192] bwd mins
        chamfers.append(f.mean() + bv.mean())
    return np.float32(np.mean(chamfers))


def kernel(pred, target):
    pred = np.asarray(pred, dtype=np.float32)
    target = np.asarray(target, dtype=np.float32)
    in_maps = _aug_inputs(pred, target)
    nc = _build_program()
    results = _run_spmd(nc, in_maps)
    return np.array(_host_reduce(results), dtype=np.float32)
